# revision 2
# baseline (speedup 1.0000x reference)
"""Expert-parallel MoE (top-1 routing) kernel for 8 TRN2 NeuronCores.

Strategy (per the expert-parallel sharding hint): the 8 experts are sharded
1:1 across the 8 cores. The router is a 0.1%-of-FLOPs linear; it is computed
host-side in float64 to decide the token->expert dispatch (the all-to-all is
realized as the host->device sharding itself: each token's activations are
DMA'd only to the core owning its expert). Each core then runs the dense
expert MLP  y = (silu(x @ gw.T) * (x @ up.T)) @ dw.T  over its gathered
tokens (padded to a uniform capacity C) with fp32 PSUM accumulation.

Layout: everything on device is kept "activation-transposed" so all three
matmuls contract over the partition dimension with zero on-device transposes:
  g_T[i_tile] = sum_k gwT[k, i].T @ x_T[k]      (psum [128(I), C])
  a_T = silu(g_T) * u_T                          (sbuf bf16)
  y_T[m_tile] += dwT[i, m].T @ a_T[i]            (psum [128(H), C], 22-step acc)

Precision: gate/up weights are stored as fp8-e3m4 (power-of-two pre-scale,
descale folded exactly into the silu scale and the DVE multiply), halving
their HBM traffic; down weights and activations stay bf16. The PE runs
mixed-dtype matmuls (fp8 stationary, bf16 moving) at the bf16 rate.

Schedule (v2, from trace analysis of v1):
  * v1 lost ~16us to the PE HAM clock-gate: the PE ran at 1.2 GHz until
    t=21.7us because serial DMA-trigger generation (~610ns per
    DMA_DIRECT2D on the SP engine) starved the stream early, and the
    resulting PE gaps kept resetting the HAM busy-window.
  * v2 issues a few garbage warm-up matmuls at body start (PE busy from
    t0 -> HAM un-throttles at ~t0+3.4us), and restructures the weight
    stream into 16 consumption-ordered triggers on one SP HWDGE queue
    with ONE counting semaphore (threshold waits). Chunks ramp small->
    large so the first real matmul can start ~2us into the body while
    trigger generation (~0.6us each) proceeds ahead of the 410 GB/s
    in-order stream.
  * y tail: DVE copies psum m0-3, ACT copies m4-7; four small y DMAs
    (m0-1/m2-3 on the SP queue, m4-5/m6-7 on the ACT queue) chase the
    copies so the final transfer is off the critical path ASAP.

Engine streams:
  SP  : x (2 pieces) + w8/wd chunks in PE-consumption order, y DMA m0-3
  PE  : warm-up MMs; per i: 8 g-MMs, 8 u-MMs, then 8 y-MMs of i-Y_LAG
  ACT : per i: silu(g)->sbuf (with 1/Sg descale); tail: psum->sbuf copies
        m4-7 + y DMA m4-7 on ACT's own HWDGE queue
  DVE : per i: a_T[i] = silu_g * u' * (1/Su) (bf16); tail: copies m0-3
"""

import math

import numpy as np
import ml_dtypes
from contextlib import ExitStack

import concourse.bass as bass
import concourse.mybir as mybir
from concourse.alu_op_type import AluOpType
from concourse.bass_utils import run_bass_kernel_spmd

S, B, H, I, E = 512, 2, 1024, 2816, 8
KT, IT, MT = H // 128, I // 128, H // 128  # 8, 22, 8
_BF = mybir.dt.bfloat16
_F8 = mybir.dt.float8e3  # e3m4
_F32 = mybir.dt.float32

GU_FP8 = True  # gate/up weights in fp8-e3m4 (halves their HBM bytes)

# CoreSim-only: gate the PE warm-up matmuls on a memset of their input so
# the simulator's uninitialized-read checker stays quiet. On hardware the
# warm-up reads garbage SBUF on purpose (results are discarded), and waiting
# would delay the clock ramp.
SIM_WARMUP_WAIT = False

Y_LAG = 2  # how many i-tiles the down-projection matmuls trail gate/up
WARMUP = 4  # N=512 garbage matmuls at body start: PE busy while x/w land,
#             so the HAM clock-gate un-throttles ~3.4us after body start
NO_GPSIMD_DRAIN = True  # skip the idle GpSimd engine's costly exit drain

# Weight-stream chunk plans, in i-tiles. Trigger generation costs ~610ns
# per DMA instruction on SP, so chunks ramp 1->5 tiles: small first chunks
# let the PE start early; later chunks amortize the trigger cost (each
# must cover >~0.61us of PE consumption = ~1 i-tile).
W8_PLAN = [(0, 1), (1, 2), (2, 3), (3, 5), (5, 8), (8, 12), (12, 17), (17, 22)]
WD_PLAN = [(0, 2), (2, 4), (4, 7), (7, 11), (11, 16), (16, 22)]
# Interleave of the SP trigger stream, in exact PE-consumption order.
# 'xA' = x k-tiles 0-3, 'xB' = k-tiles 4-7; ints index W8_PLAN/WD_PLAN.
SP_ORDER = [
    "xA", ("w8", 0), "xB", ("w8", 1),
    ("w8", 2), ("wd", 0),
    ("w8", 3), ("wd", 1),
    ("w8", 4), ("wd", 2),
    ("w8", 5), ("wd", 3),
    ("w8", 6), ("wd", 4),
    ("w8", 7), ("wd", 5),
]


def _plan_key():
    return (tuple(W8_PLAN), tuple(WD_PLAN), tuple(map(str, SP_ORDER)))


_nc_cache: dict = {}


def _build(C: int, inv_sg: float, inv_su: float) -> bass.Bass:
    """One-core program; SPMD across 8 cores (same shapes, per-core data)."""
    nc = bass.Bass()
    GUW = 2 * KT * 128  # gate|up cols per i-tile (2048)
    DW = MT * 128  # down cols per i-tile (1024)
    xt = nc.dram_tensor("xt", [128, KT * C], _BF, kind="ExternalInput")
    w8t = nc.dram_tensor("w8t", [128, IT * GUW], _F8, kind="ExternalInput")
    wdt = nc.dram_tensor("wdt", [128, IT * DW], _BF, kind="ExternalInput")
    yt = nc.dram_tensor("yt", [128, MT * C], _BF, kind="ExternalOutput")

    assert C + 256 <= 512, "two y slices must fit one PSUM bank"

    # threshold (in q_sem counts of 16) after which each chunk has landed
    w8_thr = [None] * IT
    wd_thr = [None] * IT
    x_thr = {"xA": None, "xB": None}
    for n, item in enumerate(SP_ORDER):
        thr = 16 * (n + 1)
        if item == "xA":
            x_thr["xA"] = thr
        elif item == "xB":
            x_thr["xB"] = thr
        else:
            kind, ci = item
            lo, hi = (W8_PLAN if kind == "w8" else WD_PLAN)[ci]
            for i in range(lo, hi):
                (w8_thr if kind == "w8" else wd_thr)[i] = thr
    assert all(t is not None for t in w8_thr + wd_thr)
    assert x_thr["xA"] is not None and x_thr["xB"] is not None
    # consumption order (i0 gate/up .. then y trailing) must see monotone
    # thresholds so the tensor stream's single running wait stays valid
    assert x_thr["xA"] < w8_thr[0] < x_thr["xB"] <= w8_thr[1]

    with ExitStack() as ctx:
        x_sb = ctx.enter_context(nc.sbuf_tensor([128, KT * C], _BF))
        w8_sb = ctx.enter_context(nc.sbuf_tensor([128, IT * GUW], _F8))
        wd_sb = ctx.enter_context(nc.sbuf_tensor([128, IT * DW], _BF))
        sg_sb = ctx.enter_context(nc.sbuf_tensor([128, IT * C], _F32))
        a_sb = ctx.enter_context(nc.sbuf_tensor([128, IT * C], _BF))
        # y writeback in bf16: halves the tail DMA and doubles copy rate
        # (costs ~0.2% extra output quantization, well inside the budget)
        y_sb = ctx.enter_context(nc.sbuf_tensor([128, MT * C], _BF))
        # every PSUM tensor is one full 2 KiB bank ([128, 512] f32): matmul
        # outputs must not cross bank boundaries, and the bump allocator
        # would otherwise pack tensors across banks
        g_ps = [
            ctx.enter_context(nc.psum_tensor(f"g_ps{j}", [128, 512], _F32))
            for j in range(2)
        ]
        u_ps = [
            ctx.enter_context(nc.psum_tensor(f"u_ps{j}", [128, 512], _F32))
            for j in range(2)
        ]
        y_ps = [
            ctx.enter_context(nc.psum_tensor(f"y_ps{j}", [128, 512], _F32))
            for j in range(4)
        ]

        def yslice(m):
            return y_ps[m // 2][:, (m % 2) * 256 : (m % 2) * 256 + C]

        def gw_tile(i, k):
            base = i * GUW
            return w8_sb[:, base + k * 128 : base + (k + 1) * 128]

        def uw_tile(i, k):
            base = i * GUW + KT * 128
            return w8_sb[:, base + k * 128 : base + (k + 1) * 128]

        def dw_tile(i, m):
            base = i * DW
            return wd_sb[:, base + m * 128 : base + (m + 1) * 128]

        warm_sb = ctx.enter_context(nc.sbuf_tensor([128, 512], _BF))

        q_sem = ctx.enter_context(nc.semaphore(name="q_sem"))
        ydma_sem = ctx.enter_context(nc.semaphore(name="ydma_sem"))
        warm_sem = ctx.enter_context(nc.semaphore(name="warm_sem"))
        pe_g = ctx.enter_context(nc.semaphore())
        pe_u = ctx.enter_context(nc.semaphore())
        pe_done = ctx.enter_context(nc.semaphore())
        act_sem = ctx.enter_context(nc.semaphore())
        dve_sem = ctx.enter_context(nc.semaphore())

        block = ctx.enter_context(nc.Block(no_gpsimd_drain=NO_GPSIMD_DRAIN))

        @block.sync
        def _(sync):
            # one SP HWDGE queue, all chunks in exact PE-consumption order,
            # one counting semaphore (in-order queue -> cumulative
            # thresholds). The in-order descriptor stream sustains
            # ~410 GB/s at >=2KB-per-partition descriptors.
            for item in SP_ORDER:
                if item == "xA":
                    src, dst = xt[:, : 4 * C], x_sb[:, : 4 * C]
                elif item == "xB":
                    src, dst = xt[:, 4 * C :], x_sb[:, 4 * C :]
                else:
                    kind, ci = item
                    if kind == "w8":
                        lo, hi = W8_PLAN[ci]
                        src = w8t[:, lo * GUW : hi * GUW]
                        dst = w8_sb[:, lo * GUW : hi * GUW]
                    else:
                        lo, hi = WD_PLAN[ci]
                        src = wdt[:, lo * DW : hi * DW]
                        dst = wd_sb[:, lo * DW : hi * DW]
                nc.sync.dma_start(dst, src).then_inc(q_sem, 16)
            # y writeback m0-3 on the SP queue, chasing the DVE copies
            # (dve_sem is incremented in program order by the DVE alone, so
            # >= IT+2 deterministically means copies m0-1 are done)
            nc.sync.wait_ge(dve_sem, IT + 2)
            nc.sync.dma_start(yt[:, : 2 * C], y_sb[:, : 2 * C]).then_inc(
                ydma_sem, 16
            )
            nc.sync.wait_ge(dve_sem, IT + 4)
            nc.sync.dma_start(
                yt[:, 2 * C : 4 * C], y_sb[:, 2 * C : 4 * C]
            ).then_inc(ydma_sem, 16)
            nc.sync.wait_ge(ydma_sem, 64)

        def y_block(i, stop, inc_each=False):
            for m in range(MT):
                # start=True clears has_written for the WHOLE psum bank,
                # so only the first (even) slice of each bank may set it;
                # the odd slice's first write then lands on cleared
                # has_written and overwrites cleanly.
                mm = nc.tensor.matmul(
                    yslice(m),
                    dw_tile(i, m),
                    a_sb[:, i * C : (i + 1) * C],
                    start=(i == 0 and m % 2 == 0),
                    stop=stop,
                    skip_group_check=True,
                )
                if inc_each:
                    mm.then_inc(pe_done, 1)

        @block.tensor
        def _(tensor):
            # warm-up: garbage matmuls keep the PE busy from body start so
            # the HAM clock-gate un-throttles (1.2 -> 2.4 GHz) ~3.4us in,
            # while x and the first weight chunk stream in. psum bank 0 is
            # re-initialized (start=True) by the first real matmul.
            if WARMUP:
                if SIM_WARMUP_WAIT:
                    nc.tensor.wait_ge(warm_sem, 1)
                for _ in range(WARMUP):
                    nc.tensor.matmul(
                        g_ps[0][:],
                        warm_sb[:, :128],
                        warm_sb[:],
                        start=True,
                        stop=True,
                    )
            cur_thr = 0

            def q_wait(thr):
                nonlocal cur_thr
                if thr > cur_thr:
                    nc.tensor.wait_ge(q_sem, thr)
                    cur_thr = thr

            for i in range(IT):
                pp = i % 2
                q_wait(w8_thr[i])
                if i >= 2:
                    # covers g/u psum bank reuse (mul(i-2) drained) and,
                    # for Y_LAG==2, a_T[i-2] readiness for the y-block
                    nc.tensor.wait_ge(dve_sem, i - 1)
                for k in range(KT):
                    if i == 0 and k == 4:
                        q_wait(x_thr["xB"])
                    mm = nc.tensor.matmul(
                        g_ps[pp][:, :C],
                        gw_tile(i, k),
                        x_sb[:, k * C : (k + 1) * C],
                        start=(k == 0),
                        stop=(k == KT - 1),
                    )
                mm.then_inc(pe_g, 1)
                for k in range(KT):
                    mm = nc.tensor.matmul(
                        u_ps[pp][:, :C],
                        uw_tile(i, k),
                        x_sb[:, k * C : (k + 1) * C],
                        start=(k == 0),
                        stop=(k == KT - 1),
                    )
                mm.then_inc(pe_u, 1)
                if i >= Y_LAG:
                    # y-matmuls trail gate/up by Y_LAG i-tiles so ACT->DVE
                    # chain latency never stalls the PE
                    iy = i - Y_LAG
                    if Y_LAG == 1:
                        nc.tensor.wait_ge(dve_sem, i)
                    q_wait(wd_thr[iy])
                    y_block(iy, stop=False)
            for iy in range(IT - Y_LAG, IT - 1):
                nc.tensor.wait_ge(dve_sem, iy + 1)
                q_wait(wd_thr[iy])
                y_block(iy, stop=False)
            nc.tensor.wait_ge(dve_sem, IT)
            q_wait(wd_thr[IT - 1])
            y_block(IT - 1, stop=True, inc_each=True)

        @block.scalar
        def _(scalar):
            for i in range(IT):
                pp = i % 2
                nc.scalar.wait_ge(pe_g, i + 1)
                nc.scalar.activation(
                    sg_sb[:, i * C : (i + 1) * C],
                    g_ps[pp][:, :C],
                    mybir.ActivationFunctionType.Silu,
                    scale=inv_sg,
                ).then_inc(act_sem, 1)
            # tail: psum->sbuf copies m4-7 + y DMA on ACT's own HWDGE queue
            for m in range(4, MT):
                nc.scalar.wait_ge(pe_done, m + 1)
                nc.scalar.copy(y_sb[:, m * C : (m + 1) * C], yslice(m))
                if m in (5, 7):
                    # no explicit wait: copies precede the DMA in ACT's
                    # in-order stream, and HWDGE descriptor generation
                    # happens at instruction execution time
                    nc.scalar.dma_start(
                        yt[:, (m - 1) * C : (m + 1) * C],
                        y_sb[:, (m - 1) * C : (m + 1) * C],
                    ).then_inc(ydma_sem, 16)

        @block.vector
        def _(vector):
            if SIM_WARMUP_WAIT:
                nc.vector.memset(warm_sb[:], 0.0).then_inc(warm_sem, 1)
            for i in range(IT):
                pp = i % 2
                nc.vector.wait_ge(act_sem, i + 1)
                nc.vector.wait_ge(pe_u, i + 1)
                # a = (u' * 1/Su) * silu_g   (1/Su is a power of two)
                nc.vector.scalar_tensor_tensor(
                    a_sb[:, i * C : (i + 1) * C],
                    u_ps[pp][:, :C],
                    inv_su,
                    sg_sb[:, i * C : (i + 1) * C],
                    AluOpType.mult,
                    AluOpType.mult,
                ).then_inc(dve_sem, 1)
            for m in range(4):
                nc.vector.wait_ge(pe_done, m + 1)
                nc.vector.tensor_copy(
                    y_sb[:, m * C : (m + 1) * C], yslice(m)
                ).then_inc(dve_sem, 1)

    return nc


def _bf(x):
    return np.ascontiguousarray(x).astype(ml_dtypes.bfloat16)


def _pow2_scale(absmax: float, dt) -> float:
    fmax = float(ml_dtypes.finfo(dt).max)
    return 2.0 ** math.floor(math.log2((fmax * 0.5) / absmax))


def run(hidden_states, router_w, gate_w, up_w, down_w, trace=False):
    h = np.asarray(hidden_states, dtype=np.float32)
    rw = np.asarray(router_w, dtype=np.float32)
    gw = np.asarray(gate_w, dtype=np.float32)
    uw = np.asarray(up_w, dtype=np.float32)
    dw = np.asarray(down_w, dtype=np.float32)

    T = S * B
    hf = h.reshape(T, H)
    logits = hf.astype(np.float64) @ rw.astype(np.float64).T
    ids = logits.argmax(-1)
    idx = [np.where(ids == e)[0] for e in range(E)]
    maxc = max(len(s) for s in idx)
    C = max(128, -(-maxc // 4) * 4)

    sg = _pow2_scale(float(np.abs(gw).max()), ml_dtypes.float8_e3m4)
    su = _pow2_scale(float(np.abs(uw).max()), ml_dtypes.float8_e3m4)

    key = (C, sg, su, Y_LAG, WARMUP, NO_GPSIMD_DRAIN, _plan_key())
    if key not in _nc_cache:
        _nc_cache[key] = _build(C, 1.0 / sg, 1.0 / su)
    nc = _nc_cache[key]

    in_maps = []
    for e in range(E):
        sel = idx[e]
        xp = np.zeros((C, H), np.float32)
        xp[: len(sel)] = hf[sel]
        # xt[p, k*C+c] = x[c, k*128+p]
        xt = _bf(xp.reshape(C, KT, 128).transpose(2, 1, 0).reshape(128, KT * C))
        # gwt[i, p, k*128+m] = gate_w[e][i*128+m, k*128+p]
        gwt = gw[e].reshape(IT, 128, KT, 128).transpose(0, 3, 2, 1).reshape(IT, 128, KT * 128)
        uwt = uw[e].reshape(IT, 128, KT, 128).transpose(0, 3, 2, 1).reshape(IT, 128, KT * 128)
        # dwt[i, p, m*128+mm] = down_w[e][m*128+mm, i*128+p]
        dwt = dw[e].reshape(MT, 128, IT, 128).transpose(2, 3, 0, 1).reshape(IT, 128, MT * 128)
        gu = np.concatenate([gwt * sg, uwt * su], axis=2)  # [IT,128,2048]
        w8 = np.ascontiguousarray(
            gu.transpose(1, 0, 2).reshape(128, IT * 2 * KT * 128)
        ).astype(ml_dtypes.float8_e3m4)
        wdv = _bf(dwt.transpose(1, 0, 2).reshape(128, IT * MT * 128))
        in_maps.append({"xt": xt, "w8t": w8, "wdt": wdv})

    res = run_bass_kernel_spmd(nc, in_maps, core_ids=list(range(E)), trace=trace)

    out = np.zeros((T, H), np.float32)
    for e in range(E):
        ytv = np.asarray(res.results[e]["yt"]).astype(np.float32)
        # y[c, m*128+p] = yt[p, m*C+c]
        y = ytv.reshape(128, MT, C).transpose(2, 1, 0).reshape(C, H)
        out[idx[e]] = y[: len(idx[e])]
    return out.reshape(S, B, H), res


def kernel(**inputs) -> np.ndarray:
    out, _ = run(**inputs)
    return out
